# revision 1
# baseline (speedup 1.0000x reference)
"""Multi-head attention Bass/Tile kernel for TRN2, 8-core SPMD.

Sharding: core c handles batch b = c//2, query-half qh = c%2. The host
rotates the token axis per core so query rows sit at [0:TQ] (attention is
key-permutation invariant), and gathers the unmasked keys (mask compaction)
so K/V projection + attention only touch TK <= T key tokens.

Each core: Q proj for its TQ query rows, K/V proj for the TK compacted
keys of its batch (duplicated within the batch pair), attention (softmax
without max-subtraction — scores are O(3) here; padded keys get an exp
bias of -1e30), out-proj, residual + LayerNorm.
Output per core: [TQ, D] f32 slice; host assembles [T, B, D].

Matmul layouts (out = lhsT.T @ rhs, contraction on partitions):
  QT/KT [F, *] bf16 : lhsT=w*T [D,F] chunks, rhs=hT* [D,*] chunks
  V     [TK, F] bf16: lhsT=hTk chunk [D, t128], rhs=wvT [D, F]
  S^T   [j, (h0 i512 | h1 i512)] psum (2 banks): row-tiled head pair
  exp   one ACT op per j-tile: [128, 1024], bias=maskbias per-partition
  PV+den [d0:64|den 64:128, i] psum: lhsT=V[j,64]@(0,0) + ones[j,64]@(0,64)
  O     [t, D] psum : lhsT=AVT [f, t128], rhs=woT [f, D]
"""
import numpy as np
import ml_dtypes

import concourse.bass as bass
import concourse.tile as tile
from concourse import bacc, mybir

F32 = mybir.dt.float32
BF16 = mybir.dt.bfloat16
AF = mybir.ActivationFunctionType
ALU = mybir.AluOpType

NEG_BIG = -1.0e30


def _pin_act_tables():
    """Force every ACT func we use (Exp, Ln, Square, Copy) to resolve to
    the single `natural_log_exp_and_others` table set, so the kernel does
    exactly one ACT_TABLE_LOAD instead of thrashing (~2.6us per switch).
    Preserves dict order (set ids are positional)."""
    import concourse.hw_specs as hw_specs
    if getattr(hw_specs, "_mha_tables_pinned", False):
        return
    orig = hw_specs.get_activation_tables

    def patched(module_arch):
        tabs = orig(module_arch)
        pin = "natural_log_exp_and_others"
        if pin in tabs:
            pinned_funcs = tabs[pin]
            for name, fns in tabs.items():
                if name != pin:
                    tabs[name] = fns - pinned_funcs
        return tabs

    hw_specs.get_activation_tables = patched
    import concourse.bacc as bacc_mod
    bacc_mod.get_activation_tables = patched
    hw_specs._mha_tables_pinned = True


def _chunks(total, step):
    out = []
    off = 0
    while off < total:
        out.append((off, min(step, total - off)))
        off += step
    return out


def build_nc(T, TQ, TK, D, NH, DH, n_cores=8, debug=False):
    """Build the single-core SPMD Bass program. TK = compacted key count."""
    F = NH * DH
    DC = D // 128        # D contraction chunks
    FC = F // 128        # feature chunks (2 heads per chunk, DH=64)
    KC = TK // 128       # key tiles
    TT = TQ // 128       # query t-tiles
    ICS = min(512, TQ)   # i-chunk size
    ICN = TQ // ICS
    FS = min(512, F)
    DS = min(512, D)
    assert DH == 64 and F % 128 == 0 and D % 128 == 0
    assert TQ % 128 == 0 and TK % 128 == 0

    _pin_act_tables()
    nc = bacc.Bacc("TRN2", target_bir_lowering=False, debug=debug,
                   num_devices=n_cores)

    # ---- DRAM I/O ----
    hTq_d = nc.dram_tensor("hTq", [DC * 128, TQ], BF16, kind="ExternalInput")
    hTk_d = nc.dram_tensor("hTk", [DC * 128, TK], BF16, kind="ExternalInput")
    hq_d = nc.dram_tensor("hq", [TQ, D], F32, kind="ExternalInput")
    wqT_d = nc.dram_tensor("wqT", [DC * 128, F], BF16, kind="ExternalInput")
    wkT_d = nc.dram_tensor("wkT", [DC * 128, F], BF16, kind="ExternalInput")
    wvT_d = nc.dram_tensor("wvT", [DC * 128, F], BF16, kind="ExternalInput")
    woT_d = nc.dram_tensor("woT", [FC * 128, D], BF16, kind="ExternalInput")
    mb_d = nc.dram_tensor("maskbias", [128, KC], F32, kind="ExternalInput")
    g_d = nc.dram_tensor("g_rep", [128, D], F32, kind="ExternalInput")
    b_d = nc.dram_tensor("b_rep", [128, D], F32, kind="ExternalInput")
    out_d = nc.dram_tensor("out", [TQ, D], F32, kind="ExternalOutput")

    with tile.TileContext(nc) as tc:
        with (
            tc.tile_pool(name="hpool", bufs=1) as hpool,
            tc.tile_pool(name="wts", bufs=2) as wts,
            tc.tile_pool(name="acts", bufs=1) as acts,
            tc.tile_pool(name="small", bufs=1) as small,
            tc.tile_pool(name="exps", bufs=10) as expp,
            tc.tile_pool(name="epi", bufs=3) as epi,
            tc.tile_pool(name="psA", bufs=3, space="PSUM") as psA,
            tc.tile_pool(name="psB", bufs=2, space="PSUM") as psB,
        ):
            # ---- persistent SBUF tiles ----
            hTq = hpool.tile([128, DC * TQ], BF16, tag="htq")
            hTk = hpool.tile([128, DC * TK], BF16, tag="htk")
            wqT = wts.tile([128, DC * F], BF16, tag="w")
            wkT = wts.tile([128, DC * F], BF16, tag="w")
            wvT = wts.tile([128, DC * F], BF16, tag="w")
            QT = acts.tile([128, FC * TQ], BF16, tag="qt")
            KT = acts.tile([128, FC * TK], BF16, tag="kt")
            V = acts.tile([128, KC * F], BF16, tag="v")
            AVT = acts.tile([128, FC * TQ], BF16, tag="avt")
            ones = small.tile([128, 64], BF16, tag="ones")
            mb = small.tile([128, KC], F32, tag="mb")
            eps_t = small.tile([128, 1], F32, tag="eps")

            nc.vector.memset(ones[:], 1.0)
            nc.vector.memset(eps_t[:], 1e-5)
            nc.sync.dma_start(mb[:], mb_d[:])
            # DMA in consumption order: the first QT matmul chain needs
            # wqT[dc=0] + hTq[dc=0] first; V's wvT comes last.
            for dc in range(DC):
                nc.sync.dma_start(wqT[:, dc * F:(dc + 1) * F],
                                  wqT_d[dc * 128:(dc + 1) * 128, :])
                nc.sync.dma_start(hTq[:, dc * TQ:(dc + 1) * TQ],
                                  hTq_d[dc * 128:(dc + 1) * 128, :])
            for dc in range(DC):
                nc.sync.dma_start(wkT[:, dc * F:(dc + 1) * F],
                                  wkT_d[dc * 128:(dc + 1) * 128, :])
                nc.sync.dma_start(hTk[:, dc * TK:(dc + 1) * TK],
                                  hTk_d[dc * 128:(dc + 1) * 128, :])
            for dc in range(DC):
                nc.sync.dma_start(wvT[:, dc * F:(dc + 1) * F],
                                  wvT_d[dc * 128:(dc + 1) * 128, :])

            # ---- stage 1: projections ----
            for p in range(FC):     # QT chunks [128f, TQ]
                for t0, tn in _chunks(TQ, 512):
                    ps = psA.tile([128, tn], F32, tag="A")
                    for dc in range(DC):
                        nc.tensor.matmul(
                            ps[:],
                            wqT[:, dc * F + p * 128: dc * F + (p + 1) * 128],
                            hTq[:, dc * TQ + t0: dc * TQ + t0 + tn],
                            start=(dc == 0), stop=(dc == DC - 1))
                    nc.vector.tensor_copy(
                        QT[:, p * TQ + t0: p * TQ + t0 + tn], ps[:])
            for p in range(FC):     # KT chunks [128f, TK]
                for t0, tn in _chunks(TK, 512):
                    ps = psA.tile([128, tn], F32, tag="A")
                    for dc in range(DC):
                        nc.tensor.matmul(
                            ps[:],
                            wkT[:, dc * F + p * 128: dc * F + (p + 1) * 128],
                            hTk[:, dc * TK + t0: dc * TK + t0 + tn],
                            start=(dc == 0), stop=(dc == DC - 1))
                    nc.scalar.copy(
                        KT[:, p * TK + t0: p * TK + t0 + tn], ps[:])
            def emit_v_chain(jc, f0, fn):
                ps = psA.tile([128, fn], F32, tag="A")
                for dc in range(DC):
                    nc.tensor.matmul(
                        ps[:],
                        hTk[:, dc * TK + jc * 128: dc * TK + (jc + 1) * 128],
                        wvT[:, dc * F + f0: dc * F + f0 + fn],
                        start=(dc == 0), stop=(dc == DC - 1))
                nc.vector.tensor_copy(
                    V[:, jc * F + f0: jc * F + f0 + fn], ps[:])

            for jc in range(KC):
                for f0, fn in _chunks(F, FS):
                    emit_v_chain(jc, f0, fn)

            # late loads (overlap with attention)
            woT = wts.tile([128, FC * D], BF16, tag="w")
            for fc_ in range(FC):
                nc.sync.dma_start(woT[:, fc_ * D:(fc_ + 1) * D],
                                  woT_d[fc_ * 128:(fc_ + 1) * 128, :])
            g_re = small.tile([128, D], F32, tag="g")
            b_re = small.tile([128, D], F32, tag="b")
            nc.sync.dma_start(g_re[:], g_d[:])
            nc.sync.dma_start(b_re[:], b_d[:])

            # ---- stage 2+3 interleaved: attention, then out-proj+LN
            # for each query i-chunk so the LN tail overlaps attention ----
            def oln_tile(tt):
                x = epi.tile([128, D], F32, tag="x")
                hqt = epi.tile([128, D], F32, tag="hqt")
                nc.sync.dma_start(hqt[:], hq_d[tt * 128:(tt + 1) * 128, :])
                for d0, dn in _chunks(D, DS):
                    ps = psA.tile([128, dn], F32, tag="A")
                    for fc_ in range(FC):
                        nc.tensor.matmul(
                            ps[:],
                            AVT[:, fc_ * TQ + tt * 128: fc_ * TQ + (tt + 1) * 128],
                            woT[:, fc_ * D + d0: fc_ * D + d0 + dn],
                            start=(fc_ == 0), stop=(fc_ == FC - 1))
                    nc.vector.tensor_tensor(
                        x[:, d0:d0 + dn], ps[:],
                        hqt[:, d0:d0 + dn], op=ALU.add)
                stats = epi.tile([128, 4], F32, tag="stats")
                xc = epi.tile([128, D], F32, tag="xc")
                # mean-sum on ACT (Identity+accum; xc is scratch here)
                nc.scalar.activation(xc[:], x[:], AF.Identity,
                                     accum_out=stats[:, 0:1])
                negmu = stats[:, 1:2]
                nc.vector.tensor_scalar(negmu, stats[:, 0:1], -1.0 / D, None,
                                        op0=ALU.mult)
                # xc = x - mu on ACT (Identity with per-partition bias)
                nc.scalar.activation(xc[:], x[:], AF.Identity, bias=negmu)
                # var-sum on ACT (Square+accum; x is dead scratch)
                nc.scalar.activation(x[:], xc[:], AF.Square,
                                     accum_out=stats[:, 2:3])
                var = stats[:, 3:4]
                nc.vector.tensor_scalar(var, stats[:, 2:3], 1.0 / D, None,
                                        op0=ALU.mult)
                # rstd = exp(-0.5*ln(var+eps)): Ln+Exp live in one ACT
                # table set with the attention Exps -> no table thrash
                lnv = stats[:, 0:1]
                nc.scalar.activation(lnv, var, AF.Ln, bias=eps_t[:])
                rstd = stats[:, 1:2]
                nc.scalar.activation(rstd, lnv, AF.Exp, scale=-0.5)
                nc.vector.scalar_tensor_tensor(xc[:], xc[:], rstd, g_re[:],
                                               op0=ALU.mult, op1=ALU.mult)
                nc.vector.tensor_tensor(xc[:], xc[:], b_re[:], op=ALU.add)
                nc.sync.dma_start(out_d[tt * 128:(tt + 1) * 128, :], xc[:])

            for ic in range(ICN):
                io = ic * ICS
                for hp in range(FC):
                    h0, h1 = 2 * hp, 2 * hp + 1
                    pvP = psB.tile([128, ICS], F32, tag="pv")
                    pvD = psB.tile([128, ICS], F32, tag="pv")
                    for jc in range(KC):
                        # S^T pair: [j, h0-i | h1-i] across 2 psum banks
                        s = psA.tile([128, 2 * ICS], F32, tag="A")
                        nc.tensor.matmul(
                            s[:, 0:ICS],
                            KT[0:64, hp * TK + jc * 128: hp * TK + (jc + 1) * 128],
                            QT[0:64, hp * TQ + io: hp * TQ + io + ICS],
                            start=True, stop=True, tile_position=(0, 0))
                        nc.tensor.matmul(
                            s[:, ICS:2 * ICS],
                            KT[64:128, hp * TK + jc * 128: hp * TK + (jc + 1) * 128],
                            QT[64:128, hp * TQ + io: hp * TQ + io + ICS],
                            start=True, stop=True, tile_position=(64, 0))
                        e = expp.tile([128, 2 * ICS], BF16, tag="e")
                        nc.scalar.activation(e[:], s[:], AF.Exp,
                                             bias=mb[:, jc:jc + 1])
                        st, sp = (jc == 0), (jc == KC - 1)
                        nc.tensor.matmul(
                            pvP[0:64, :],
                            V[:, jc * F + h0 * DH: jc * F + (h0 + 1) * DH],
                            e[:, 0:ICS], start=st, stop=sp,
                            tile_position=(0, 0), skip_group_check=True)
                        nc.tensor.matmul(
                            pvP[64:128, :],
                            V[:, jc * F + h1 * DH: jc * F + (h1 + 1) * DH],
                            e[:, ICS:2 * ICS], start=st, stop=sp,
                            tile_position=(0, 64), skip_group_check=True)
                        nc.tensor.matmul(
                            pvD[0:64, :], ones[:, 0:64],
                            e[:, 0:ICS], start=st, stop=sp,
                            tile_position=(0, 0), skip_group_check=True)
                        nc.tensor.matmul(
                            pvD[64:128, :], ones[:, 0:64],
                            e[:, ICS:2 * ICS], start=st, stop=sp,
                            tile_position=(0, 64), skip_group_check=True)
                    # normalize -> AVT: copy PV/den to SBUF fast (frees
                    # the PSUM banks for the next head pair), then one
                    # reciprocal + one multiply covering both heads.
                    pvPc = epi.tile([128, ICS], F32, tag="rec")
                    pvDc = epi.tile([128, ICS], F32, tag="rec")
                    nc.vector.tensor_copy(pvPc[:], pvP[:])
                    nc.vector.tensor_copy(pvDc[:], pvD[:])
                    nc.vector.reciprocal(pvDc[:], pvDc[:])
                    nc.vector.tensor_tensor(
                        AVT[:, hp * TQ + io: hp * TQ + io + ICS],
                        pvPc[:], pvDc[:], op=ALU.mult)

                # out-proj + residual + LayerNorm for this ic's t-tiles
                for tt in range(io // 128, (io + ICS) // 128):
                    oln_tile(tt)

    nc.compile()
    return nc


def choose_tk(attn_mask):
    """Compacted key count: max unmasked count over batches, ceil to 128."""
    m = np.asarray(attn_mask)
    counts = (~m).sum(axis=0)
    tk = int(((int(counts.max()) + 127) // 128) * 128)
    return max(tk, 128)


def host_prep_core(c, tk, h, attn_mask, wq, wkv, wo, ln_g, ln_b, NH=16, DH=64):
    """Build the per-core input map (numpy) for core c."""
    T, B, D = h.shape
    F = NH * DH
    TQ = T // 2
    KC = tk // 128
    b, qh = c // 2, c % 2
    bf = ml_dtypes.bfloat16
    hb = np.roll(np.asarray(h[:, b, :], dtype=np.float32), -qh * TQ, axis=0)
    maskb = np.roll(np.asarray(attn_mask[:, b]), -qh * TQ)
    idx = np.nonzero(~maskb)[0]
    nk = idx.shape[0]
    assert nk <= tk
    idxp = np.concatenate([idx, np.zeros(tk - nk, np.int64)])
    scale = 1.0 / np.sqrt(DH)
    hbT = np.ascontiguousarray(hb.T).astype(bf)             # [D, T]
    m = {}
    m["hTq"] = np.ascontiguousarray(hbT[:, :TQ])
    m["hTk"] = np.ascontiguousarray(hbT[:, idxp])
    m["hq"] = np.ascontiguousarray(hb[:TQ])                 # [TQ, D] f32
    m["wqT"] = np.ascontiguousarray(wq.T * scale).astype(bf)
    m["wkT"] = np.ascontiguousarray(wkv[:F].T).astype(bf)
    m["wvT"] = np.ascontiguousarray(wkv[F:].T).astype(bf)
    m["woT"] = np.ascontiguousarray(wo.T).astype(bf)
    mbias = np.full(tk, NEG_BIG, np.float32)
    mbias[:nk] = 0.0
    m["maskbias"] = np.ascontiguousarray(mbias.reshape(KC, 128).T)
    m["g_rep"] = np.ascontiguousarray(
        np.broadcast_to(np.asarray(ln_g, np.float32), (128, D)))
    m["b_rep"] = np.ascontiguousarray(
        np.broadcast_to(np.asarray(ln_b, np.float32), (128, D)))
    return m

# ======================================================================
# Host-side runner: shard, compile (cached), execute on 8 cores, gather.
# ======================================================================
_NC_CACHE = {}
LAST_RESULT = None  # BassKernelResults of the most recent kernel() call


def _get_nc(T, TQ, TK, D, NH, DH):
    key = (T, TQ, TK, D, NH, DH)
    if key not in _NC_CACHE:
        _NC_CACHE[key] = build_nc(T, TQ, TK, D, NH, DH, n_cores=8, debug=False)
    return _NC_CACHE[key]


def kernel(h, attn_mask, wq, wkv, wo, ln_g, ln_b):
    """Full-input MultiHeadAttn forward on 8 NeuronCores.

    h: [T, B, D] f32; attn_mask: [T, B] bool (True = masked key);
    wq: [F, D]; wkv: [2F, D]; wo: [D, F]; ln_g/ln_b: [D].
    Returns [T, B, D] f32 = layer_norm(h + attn(h)).
    """
    from concourse.bass_utils import run_bass_kernel_spmd
    global LAST_RESULT

    h = np.asarray(h)
    attn_mask = np.asarray(attn_mask)
    wq = np.asarray(wq, np.float32)
    wkv = np.asarray(wkv, np.float32)
    wo = np.asarray(wo, np.float32)
    ln_g = np.asarray(ln_g, np.float32)
    ln_b = np.asarray(ln_b, np.float32)

    T, B, D = h.shape
    NH = 16
    DH = wq.shape[0] // NH
    assert 2 * B == 8, "sharding assumes batch 4 over 8 cores"
    TQ = T // 2
    TK = min(choose_tk(attn_mask), T)

    nc = _get_nc(T, TQ, TK, D, NH, DH)
    in_maps = [host_prep_core(c, TK, h, attn_mask, wq, wkv, wo, ln_g, ln_b,
                              NH=NH, DH=DH) for c in range(8)]
    res = run_bass_kernel_spmd(nc, in_maps, core_ids=list(range(8)))
    LAST_RESULT = res

    out = np.empty((T, B, D), np.float32)
    for c in range(8):
        b, qh = c // 2, c % 2
        out[qh * TQ:(qh + 1) * TQ, b, :] = res.results[c]["out"]
    return out

